# revision 6
# baseline (speedup 1.0000x reference)
"""CyclicalAttention Trainium2 kernel — 8-core SPMD, head-sharded, v2.

Sharding: 16 heads / 8 cores = 2 heads per core (both batches on every
core).  Per core (Megatron-style):
  - column-parallel Q/K/V projections for its 128-dim head slice
  - full attention for its 2 heads x 2 batches
  - row-parallel slice of the output projection -> partial y
Host sums the 8 partial outputs and adds bo (+ bv folded through wo).

v2 schedule (vs v1):
  - x^T and W are host-packed into SBUF layout so x streams in 5 large
    column-block DMAs; Q/K projections and the first attention unit
    start at ~4us instead of ~25us
  - per-batch pipeline: b0 proj -> b0 attention (draining V(b0), the
    b1 projections and finished oproj chunks into PE slack) -> b1
  - oproj chunks enqueue per (batch, q-half) as soon as both heads of
    that q-half are normalized, shrinking the tail
  - DMA triggers ride idle queues (sync/vector/gpsimd); the scalar
    engine keeps its sequencer for exp
"""

import math

import numpy as np
import ml_dtypes

D_MODEL = 1024
N_HEADS = 16
HEAD_DIM = 64
B, S = 2, 2048
EPS = 1e-12
N_CORES = 8
HPC = N_HEADS // N_CORES          # heads per core = 2
DC = HPC * HEAD_DIM               # per-core model-dim slice = 128
NSEQ = B * S                      # 4096
P = 128
BF16 = ml_dtypes.bfloat16

_CACHE = {}


def _build_module(repeat=1, opts=None):
    opts = opts or {}
    PS_SC = opts.get("ps_sc", 2)
    PS_PV = opts.get("ps_pv", 1)
    PS_DR = opts.get("ps_dr", 2)      # >0: separate small pool for drains
    PREFIX = opts.get("prefix", "full8")
    VJIT = opts.get("vjit", False)
    TAILCP = opts.get("tailcp", True)  # scalar copies only in the tail
    EP = opts.get("ep", 6)
    YP = opts.get("yp", 6)
    import contextlib

    import concourse.bacc as bacc
    import concourse.mybir as mybir
    import concourse.tile as tile
    from concourse import library_config

    f32 = mybir.dt.float32
    bf16 = mybir.dt.bfloat16
    Exp = mybir.ActivationFunctionType.Exp
    mult = mybir.AluOpType.mult
    add = mybir.AluOpType.add

    nc = bacc.Bacc(
        "TRN2",
        target_bir_lowering=False,
        debug=False,
        enable_asserts=False,
        num_devices=N_CORES,
    )

    xh_d = nc.dram_tensor("xh", [P, 8, NSEQ], bf16, kind="ExternalInput").ap()
    wq_d = nc.dram_tensor("wq_p", [P, 8, DC], bf16, kind="ExternalInput").ap()
    wk_d = nc.dram_tensor("wk_p", [P, 8, DC], bf16, kind="ExternalInput").ap()
    wv_d = nc.dram_tensor("wv_p", [P, 8, DC], bf16, kind="ExternalInput").ap()
    wo_d = nc.dram_tensor("wo_t", [DC, D_MODEL], bf16, kind="ExternalInput").ap()
    qaug_d = nc.dram_tensor("qaug", [HPC, NSEQ], bf16, kind="ExternalInput").ap()
    kaug_d = nc.dram_tensor("kaug", [HPC, NSEQ], bf16, kind="ExternalInput").ap()
    bq8_d = nc.dram_tensor("bq8", [DC, 1], f32, kind="ExternalInput").ap()
    bk_d = nc.dram_tensor("bk", [DC, 1], f32, kind="ExternalInput").ap()
    yt_d = nc.dram_tensor("yt", [D_MODEL, NSEQ], bf16, kind="ExternalOutput").ap()

    KT = D_MODEL // P   # 8 contraction tiles for the projections
    SCT = S // P        # 16 k-tiles per (b, h) in attention

    with tile.TileContext(nc) as tc:
        with (
            tc.tile_pool(name="consts", bufs=1) as consts,
            tc.tile_pool(name="xtp", bufs=1) as xtp,
            tc.tile_pool(name="acts", bufs=1) as acts,
            tc.tile_pool(name="ep", bufs=EP) as ep,
            tc.tile_pool(name="rp", bufs=2) as rp,
            tc.tile_pool(name="yp", bufs=YP) as yp,
            tc.tile_pool(name="ps_sc", bufs=PS_SC, space="PSUM") as ps_sc,
            tc.tile_pool(name="ps_pv", bufs=PS_PV, space="PSUM") as ps_pv,
            tc.tile_pool(name="ps_dr", bufs=max(PS_DR, 1), space="PSUM") as ps_dr,
            tc.For_i(0, repeat, 1) if repeat > 1 else contextlib.nullcontext(),
        ):
            nc.gpsimd.load_library(library_config.attn)

            # ---- weights / biases / augs ----
            wq_sb = consts.tile([P, KT, DC], bf16)
            wk_sb = consts.tile([P, KT, DC], bf16)
            wv_sb = consts.tile([P, KT, DC], bf16)
            wo_sb = consts.tile([DC, D_MODEL], bf16)
            bq8_sb = consts.tile([DC, 1], f32)
            bk_sb = consts.tile([DC, 1], f32)


            # x^T in SBUF layout [128, t, n]; streamed in column blocks so
            # the b0 projections start as soon as the first block lands
            xall = xtp.tile([P, KT, NSEQ], bf16, tag="xall", name="xall")
            nc.scalar.dma_start(wq_sb[:], wq_d)
            nc.sync.dma_start(xall[:, 0:4, 0:512], xh_d[:, 0:4, 0:512])
            nc.sync.dma_start(xall[:, 4:8, 0:512], xh_d[:, 4:8, 0:512])
            nc.sync.dma_start(wk_sb[:], wk_d)
            nc.scalar.dma_start(xall[:, :, 512:1024], xh_d[:, :, 512:1024])
            nc.sync.dma_start(xall[:, :, 1024:2048], xh_d[:, :, 1024:2048])
            nc.scalar.dma_start(xall[:, :, 2048:3072], xh_d[:, :, 2048:3072])
            nc.sync.dma_start(xall[:, :, 3072:4096], xh_d[:, :, 3072:4096])

            # small/late loads on the gpsimd (SWDGE) queue
            nc.gpsimd.dma_start(bq8_sb[:], bq8_d)
            nc.gpsimd.dma_start(bk_sb[:], bk_d)

            # Q^T / K^T augmented per local head: [65, 4096]
            qt_sb = [acts.tile([HEAD_DIM + 1, NSEQ], bf16, tag=f"qt{h}", name=f"qt{h}") for h in range(HPC)]
            kt_sb = [acts.tile([HEAD_DIM + 1, NSEQ], bf16, tag=f"kt{h}", name=f"kt{h}") for h in range(HPC)]
            # b0 halves of the h0 augs are needed by the first scores tile
            nc.sync.dma_start(qt_sb[0][HEAD_DIM : HEAD_DIM + 1, :S], qaug_d[0:1, :S])
            nc.scalar.dma_start(kt_sb[0][HEAD_DIM : HEAD_DIM + 1, :S], kaug_d[0:1, :S])
            nc.gpsimd.dma_start(wv_sb[:], wv_d)
            nc.gpsimd.dma_start(qt_sb[1][HEAD_DIM : HEAD_DIM + 1, :S], qaug_d[1:2, :S])
            nc.gpsimd.dma_start(kt_sb[1][HEAD_DIM : HEAD_DIM + 1, :S], kaug_d[1:2, :S])
            for h in range(HPC):
                nc.gpsimd.dma_start(qt_sb[h][HEAD_DIM : HEAD_DIM + 1, S:], qaug_d[h : h + 1, S:])
                nc.gpsimd.dma_start(kt_sb[h][HEAD_DIM : HEAD_DIM + 1, S:], kaug_d[h : h + 1, S:])
            nc.gpsimd.dma_start(wo_sb[:], wo_d)

            # V_aug: [128(k), bh, kt, 65]; col 64 = ones (denominator trick)
            v_all = acts.tile([P, B * HPC, SCT, HEAD_DIM + 1], bf16, tag="vall")
            nc.vector.memset(v_all[:, :, :, HEAD_DIM : HEAD_DIM + 1], 1.0)
            # attention output (d-major), per batch
            ao_sb = [acts.tile([DC, S], bf16, tag=f"ao{b}", name=f"ao{b}") for b in range(B)]

            # ---- projection chunk emitters (n = 512-col chunk of seq) ----
            def proj_chunk(w_sb, post, n, t0=0, t1=KT, ps_box=[None]):
                if t0 == 0:
                    if PS_DR:
                        ps_box[0] = ps_dr.tile([P, 512], f32, tag="dr", name="ps_p")
                    else:
                        ps_box[0] = ps_sc.tile([P, 1024], f32, tag="mm", name="ps_p")
                pss = ps_box[0][:, :512]
                for t in range(t0, t1):
                    nc.tensor.matmul(
                        pss,
                        w_sb[:, t, :],
                        xall[:, t, n * 512 : (n + 1) * 512],
                        start=(t == 0),
                        stop=(t == KT - 1),
                    )
                if t1 == KT:
                    post(n, pss)

            def q_post(n, pss):
                for h in range(HPC):
                    nc.vector.tensor_scalar(
                        qt_sb[h][:HEAD_DIM, n * 512 : (n + 1) * 512],
                        pss[h * HEAD_DIM : (h + 1) * HEAD_DIM, :],
                        0.125,
                        bq8_sb[h * HEAD_DIM : (h + 1) * HEAD_DIM, :],
                        mult,
                        add,
                    )

            def k_post(n, pss):
                for h in range(HPC):
                    nc.vector.tensor_scalar_add(
                        kt_sb[h][:HEAD_DIM, n * 512 : (n + 1) * 512],
                        pss[h * HEAD_DIM : (h + 1) * HEAD_DIM, :],
                        bk_sb[h * HEAD_DIM : (h + 1) * HEAD_DIM, :],
                    )

            def proj_halves(w_sb, post, n):
                box = [None]
                return [
                    lambda: proj_chunk(w_sb, post, n, 0, 4, box),
                    lambda: proj_chunk(w_sb, post, n, 4, KT, box),
                ]

            # ---- V chunks: [k, dv] layout straight from the matmul ----
            def vnat_chunk(sc):
                def emit():
                    b, kt = divmod(sc, SCT)
                    if PS_DR:
                        ps = ps_dr.tile([P, 512], f32, tag="dr", name="ps_v")
                    else:
                        ps = ps_sc.tile([P, 1024], f32, tag="mm", name="ps_v")
                    pss = ps[:, :DC]
                    for t in range(KT):
                        nc.tensor.matmul(
                            pss,
                            xall[:, t, sc * P : (sc + 1) * P],
                            wv_sb[:, t, :],
                            start=(t == 0),
                            stop=(t == KT - 1),
                        )
                    nc.vector.tensor_copy(
                        v_all[:, b * HPC : (b + 1) * HPC, kt, :HEAD_DIM],
                        pss[:, :DC],
                    )

                return emit

            # ---- output projection chunks ----
            n_yd = [0]

            def oproj_chunk(b, ec, sc2):
                def emit():
                    # in the tail the scores pool is free: alternate pools for
                    # a deeper rotation so matmuls don't wait on copies
                    if not PS_DR or (tail[0] and n_yd[0] % 2 == 0):
                        ps = ps_sc.tile([P, 1024], f32, tag="mm", name="ps_o")
                    else:
                        ps = ps_dr.tile([P, 512], f32, tag="dr", name="ps_o")
                    pss = ps[:, :512]
                    nc.tensor.matmul(
                        pss,
                        wo_sb[:, ec * P : (ec + 1) * P],
                        ao_sb[b][:, sc2 * 512 : (sc2 + 1) * 512],
                        start=True,
                        stop=True,
                    )
                    y_sb = yp.tile([P, 512], bf16, tag="y", name="y_sb")
                    # only the post-attention tail may use the scalar engine
                    # (exp still owns it mid-kernel); alternate there
                    if (not TAILCP or tail[0]) and n_yd[0] % 2 == 0:
                        nc.scalar.copy(y_sb[:], pss)
                    else:
                        nc.vector.tensor_copy(y_sb[:], pss)
                    n_yd[0] += 1
                    dma_eng = (nc.sync, nc.gpsimd)[n_yd[0] % 2]
                    dma_eng.dma_start(
                        yt_d[
                            ec * P : (ec + 1) * P,
                            b * S + sc2 * 512 : b * S + (sc2 + 1) * 512,
                        ],
                        y_sb[:],
                    )

                return emit

            pending = []
            tail = [False]

            def drain(n=1):
                for _ in range(min(n, len(pending))):
                    pending.pop(0)()

            # ---- attention per (b, h, q-half) ----
            def attn_unit(b, h, qh, vjit=False, last=False, dr=None):
                col0 = b * S
                pv = ps_pv.tile([HEAD_DIM + 1, 1024], f32, tag="pv", name="pv")
                for kt in range(SCT):
                    if vjit:
                        vnat_chunk(b * SCT + kt)()
                    drain(dr[kt % len(dr)] if dr else 1)
                    ps = ps_sc.tile([P, 1024], f32, tag="mm", name="ps_s")
                    for c in range(2):
                        q0 = col0 + qh * 1024 + c * 512
                        nc.tensor.matmul(
                            ps[:, c * 512 : (c + 1) * 512],
                            kt_sb[h][:, col0 + kt * P : col0 + (kt + 1) * P],
                            qt_sb[h][:, q0 : q0 + 512],
                            start=True,
                            stop=True,
                        )
                    e = ep.tile([P, 1024], bf16, tag="e", name="e")
                    nc.scalar.activation(e[:], ps[:], Exp)
                    for c in range(2):
                        nc.tensor.matmul(
                            pv[:, c * 512 : (c + 1) * 512],
                            v_all[:, b * HPC + h, kt, :],
                            e[:, c * 512 : (c + 1) * 512],
                            start=(kt == 0),
                            stop=(kt == SCT - 1),
                        )
                # normalize: out = pv[0:64] / pv[64], in 512-col halves so
                # the matching oproj chunks unblock sooner.  Mid-kernel the
                # pv half is first copied to SBUF (frees the PSUM accumulator
                # for the next unit ~1.5us sooner); the last unit normalizes
                # straight from PSUM.
                srcs, rbs = [], []
                for hf in range(2):
                    cl = hf * 512
                    if last:
                        src = pv[:, cl : cl + 512]
                    else:
                        pvc = rp.tile([HEAD_DIM + 1, 512], f32, tag="pvc", name="pvc")
                        nc.vector.tensor_copy(pvc[:], pv[:, cl : cl + 512])
                        src = pvc[:]
                    srcs.append(src)
                    r_sb = rp.tile([1, 512], f32, tag="r", name="r_sb")
                    nc.vector.reciprocal(r_sb[:], src[HEAD_DIM : HEAD_DIM + 1, :])
                    rb = rp.tile([HEAD_DIM, 512], f32, tag="rb", name="rb")
                    nc.gpsimd.partition_broadcast(rb[:], r_sb[:])
                    rbs.append(rb)
                for hf in range(2):
                    cl = hf * 512
                    nc.vector.tensor_tensor(
                        ao_sb[b][
                            h * HEAD_DIM : (h + 1) * HEAD_DIM,
                            qh * 1024 + cl : qh * 1024 + cl + 512,
                        ],
                        srcs[hf][:HEAD_DIM, :],
                        rbs[hf][:],
                        mult,
                    )

            # ---- schedule ----
            if PREFIX == "min3i":
                # minimal prefix: attention unit0 starts after Q0,K0,Q1;
                # V(b0) is interleaved 1:1 with the remaining b0 projection
                # halves so unit0's drain(2) covers PV's JIT needs
                proj_chunk(wq_sb, q_post, 0)
                proj_chunk(wk_sb, k_post, 0)
                proj_chunk(wq_sb, q_post, 1)
                misc = []
                for n in (1, 2, 3):
                    misc.extend(proj_halves(wk_sb, k_post, n))
                for n in (2, 3):
                    misc.extend(proj_halves(wq_sb, q_post, n))
                for kt in range(SCT):
                    pending.append(vnat_chunk(kt))
                    if kt < len(misc):
                        pending.append(misc[kt])
                pending.extend(misc[SCT:])
                for sc in range(SCT, B * SCT):
                    pending.append(vnat_chunk(sc))
                for n in range(4, 8):
                    pending.extend(proj_halves(wq_sb, q_post, n))
                    pending.extend(proj_halves(wk_sb, k_post, n))
            elif PREFIX == "min3":
                proj_chunk(wq_sb, q_post, 0)
                proj_chunk(wk_sb, k_post, 0)
                proj_chunk(wq_sb, q_post, 1)
                for n in (1, 2, 3):
                    pending.extend(proj_halves(wk_sb, k_post, n))
                for n in (2, 3):
                    pending.extend(proj_halves(wq_sb, q_post, n))
            else:
                proj_chunk(wq_sb, q_post, 0)
                proj_chunk(wk_sb, k_post, 0)
                proj_chunk(wq_sb, q_post, 1)
                proj_chunk(wk_sb, k_post, 1)
                proj_chunk(wk_sb, k_post, 2)
                proj_chunk(wk_sb, k_post, 3)
                proj_chunk(wq_sb, q_post, 2)
                proj_chunk(wq_sb, q_post, 3)
            if PREFIX != "min3i":
                # V chunks FIRST: unit0's kt loop drains vnat(b0, kt) exactly
                # one iteration ahead of the PV matmul that consumes it
                if not VJIT:
                    for sc in range(B * SCT):
                        pending.append(vnat_chunk(sc))
                for n in range(4, 8):
                    pending.extend(proj_halves(wq_sb, q_post, n))
                    pending.extend(proj_halves(wk_sb, k_post, n))

            # drain cadence per unit: early units have mandatory JIT V
            # chunks; later units smooth the remaining fill work over the
            # Act-bound attention steady state (~186ns/kt deficit)
            CAD = opts.get("cad") or [
                [1], [1], [1, 0], [1, 0],
                [1, 1, 1, 0], [1, 1, 1, 0], [1, 1, 1, 0], [1, 1, 1, 0],
            ]
            ui = 0
            for b in range(B):
                for qh in range(2):
                    for h in range(HPC):
                        first = b == 0 and qh == 0 and h == 0
                        attn_unit(b, h, qh, vjit=VJIT and (qh == 0 and h == 0),
                                  last=(b == B - 1 and qh == 1 and h == HPC - 1),
                                  dr=[2] if (PREFIX == "min3i" and first) else CAD[min(ui, 7)])
                        ui += 1
                    # both heads of this q-half done -> its oproj chunks
                    # (sc2-major so tail chunks gated on the first normalize
                    # half don't queue behind second-half chunks)
                    pending.extend(
                        oproj_chunk(b, ec, sc2)
                        for sc2 in (2 * qh, 2 * qh + 1)
                        for ec in range(D_MODEL // P)
                    )
            tail[0] = True
            drain(len(pending))

    nc.compile()
    return nc


def _get_module(repeat=1, opts=None):
    key = f"nc{repeat}{opts}"
    if key not in _CACHE:
        _CACHE[key] = _build_module(repeat, opts)
    return _CACHE[key]


def _host_prep(x, temporal_features, wq, bq, wk, bk, wv, bv, wo, bo, wc, bc, cycle_scale):
    """Shard/lay out the inputs for the 8 cores."""
    x = np.asarray(x, np.float32)
    # [d, n] -> SBUF layout [p, t, n] with d = t*128 + p
    xt = np.ascontiguousarray(
        x.reshape(NSEQ, D_MODEL).T.reshape(8, P, NSEQ).transpose(1, 0, 2)
    ).astype(BF16)

    # cycle embedding (tiny): [B, S, H] -> [B, H, S], seq-normalized
    ce = (
        np.asarray(temporal_features, np.float32).reshape(NSEQ, -1) @ np.asarray(wc, np.float32).T
        + np.asarray(bc, np.float32)
    ).reshape(B, S, N_HEADS).transpose(0, 2, 1)
    nrm = np.maximum(np.linalg.norm(ce, axis=-1, keepdims=True), EPS)
    cn = ce / nrm  # [B, H, S]
    cs = np.asarray(cycle_scale, np.float32)

    def pack_w(w_rows):  # [DC, D] slice of W -> SBUF layout [p, t, m]
        wt = np.ascontiguousarray(w_rows.T)  # [D, DC]
        return np.ascontiguousarray(
            wt.reshape(8, P, DC).transpose(1, 0, 2)
        ).astype(BF16)

    in_maps = []
    for c in range(N_CORES):
        rows = slice(c * DC, (c + 1) * DC)
        qaug = np.empty((HPC, NSEQ), np.float32)
        kaug = np.empty((HPC, NSEQ), np.float32)
        for h in range(HPC):
            gh = c * HPC + h
            for b in range(B):
                qaug[h, b * S : (b + 1) * S] = cs[gh] * cn[b, gh]
                kaug[h, b * S : (b + 1) * S] = cn[b, gh]
        in_maps.append(
            {
                "xh": xt,
                "wq_p": pack_w(np.asarray(wq, np.float32)[rows]),
                "wk_p": pack_w(np.asarray(wk, np.float32)[rows]),
                "wv_p": pack_w(np.asarray(wv, np.float32)[rows]),
                "wo_t": np.ascontiguousarray(np.asarray(wo, np.float32)[:, rows].T).astype(BF16),
                "qaug": qaug.astype(BF16),
                "kaug": kaug.astype(BF16),
                "bq8": (np.asarray(bq, np.float32)[rows] * 0.125).reshape(DC, 1).copy(),
                "bk": np.asarray(bk, np.float32)[rows].reshape(DC, 1).copy(),
            }
        )
    return in_maps


def kernel(**inputs):
    from concourse import bass_utils

    nc = _get_module()
    in_maps = _host_prep(**inputs)
    res = bass_utils.run_bass_kernel_spmd(nc, in_maps, core_ids=list(range(N_CORES)))
    yt = np.zeros((D_MODEL, NSEQ), np.float64)
    for r in res.results:
        yt += r["yt"].astype(np.float64)
    # bv is folded out of the device kernel: attn rows sum to 1, so
    # attn@(V+bv) @ wo.T = attn@V @ wo.T + bv @ wo.T
    bias = np.asarray(inputs["bo"], np.float64) + np.asarray(
        inputs["bv"], np.float64
    ) @ np.asarray(inputs["wo"], np.float64).T
    y = yt.T.reshape(B, S, D_MODEL) + bias
    return y.astype(np.float32)


# revision 8
# speedup vs baseline: 1.2667x; 1.2667x over previous
"""CyclicalAttention Trainium2 kernel — 8-core SPMD, head-sharded, v2.

Sharding: 16 heads / 8 cores = 2 heads per core (both batches on every
core).  Per core (Megatron-style):
  - column-parallel Q/K/V projections for its 128-dim head slice
  - full attention for its 2 heads x 2 batches
  - row-parallel slice of the output projection -> partial y
Host sums the 8 partial outputs and adds bo (+ bv folded through wo).

v2 schedule (vs v1):
  - x^T and W are host-packed into SBUF layout so x streams in 5 large
    column-block DMAs; Q/K projections and the first attention unit
    start at ~4us instead of ~25us
  - per-batch pipeline: b0 proj -> b0 attention (draining V(b0), the
    b1 projections and finished oproj chunks into PE slack) -> b1
  - oproj chunks enqueue per (batch, q-half) as soon as both heads of
    that q-half are normalized, shrinking the tail
  - DMA triggers ride idle queues (sync/vector/gpsimd); the scalar
    engine keeps its sequencer for exp
"""

import math

import numpy as np
import ml_dtypes

D_MODEL = 1024
N_HEADS = 16
HEAD_DIM = 64
B, S = 2, 2048
EPS = 1e-12
N_CORES = 8
HPC = N_HEADS // N_CORES          # heads per core = 2
DC = HPC * HEAD_DIM               # per-core model-dim slice = 128
NSEQ = B * S                      # 4096
P = 128
BF16 = ml_dtypes.bfloat16

_CACHE = {}


def _build_module(repeat=1, opts=None):
    opts = opts or {}
    PS_SC = opts.get("ps_sc", 2)
    PS_PV = opts.get("ps_pv", 1)
    PS_DR = opts.get("ps_dr", 2)      # >0: separate small pool for drains
    PREFIX = opts.get("prefix", "full8")
    VJIT = opts.get("vjit", False)
    TAILCP = opts.get("tailcp", True)  # scalar copies only in the tail
    EP = opts.get("ep", 6)
    YP = opts.get("yp", 6)
    import contextlib

    import concourse.bacc as bacc
    import concourse.mybir as mybir
    import concourse.tile as tile
    from concourse import library_config

    f32 = mybir.dt.float32
    bf16 = mybir.dt.bfloat16
    Exp = mybir.ActivationFunctionType.Exp
    mult = mybir.AluOpType.mult
    add = mybir.AluOpType.add

    nc = bacc.Bacc(
        "TRN2",
        target_bir_lowering=False,
        debug=False,
        enable_asserts=False,
        num_devices=N_CORES,
    )

    xh_d = nc.dram_tensor("xh", [P, 8, NSEQ], bf16, kind="ExternalInput").ap()
    wq_d = nc.dram_tensor("wq_p", [P, 8, DC], bf16, kind="ExternalInput").ap()
    wk_d = nc.dram_tensor("wk_p", [P, 8, DC], bf16, kind="ExternalInput").ap()
    wv_d = nc.dram_tensor("wv_p", [P, 8, DC], bf16, kind="ExternalInput").ap()
    wo_d = nc.dram_tensor("wo_t", [DC, D_MODEL], bf16, kind="ExternalInput").ap()
    qaug_d = nc.dram_tensor("qaug", [HPC, NSEQ], bf16, kind="ExternalInput").ap()
    kaug_d = nc.dram_tensor("kaug", [HPC, NSEQ], bf16, kind="ExternalInput").ap()
    bq8_d = nc.dram_tensor("bq8", [DC, 1], f32, kind="ExternalInput").ap()
    bk_d = nc.dram_tensor("bk", [DC, 1], f32, kind="ExternalInput").ap()
    yt_d = nc.dram_tensor("yt", [D_MODEL, NSEQ], bf16, kind="ExternalOutput").ap()

    KT = D_MODEL // P   # 8 contraction tiles for the projections
    SCT = S // P        # 16 k-tiles per (b, h) in attention

    with tile.TileContext(nc) as tc:
        with (
            tc.tile_pool(name="consts", bufs=1) as consts,
            tc.tile_pool(name="xtp", bufs=1) as xtp,
            tc.tile_pool(name="acts", bufs=1) as acts,
            tc.tile_pool(name="ep", bufs=EP) as ep,
            tc.tile_pool(name="rp", bufs=2) as rp,
            tc.tile_pool(name="yp", bufs=YP) as yp,
            tc.tile_pool(name="ps_sc", bufs=PS_SC, space="PSUM") as ps_sc,
            tc.tile_pool(name="ps_pv", bufs=PS_PV, space="PSUM") as ps_pv,
            tc.tile_pool(name="ps_dr", bufs=max(PS_DR, 1), space="PSUM") as ps_dr,
            tc.For_i(0, repeat, 1) if repeat > 1 else contextlib.nullcontext(),
        ):
            nc.gpsimd.load_library(library_config.attn)

            # ---- weights / biases / augs ----
            wq_sb = consts.tile([P, KT, DC], bf16)
            wk_sb = consts.tile([P, KT, DC], bf16)
            wv_sb = consts.tile([P, KT, DC], bf16)
            wo_sb = consts.tile([DC, D_MODEL], bf16)
            bq8_sb = consts.tile([DC, 1], f32)
            bk_sb = consts.tile([DC, 1], f32)


            # x^T in SBUF layout [128, t, n]; streamed in column blocks so
            # the b0 projections start as soon as the first block lands
            xall = xtp.tile([P, KT, NSEQ], bf16, tag="xall", name="xall")
            nc.scalar.dma_start(wq_sb[:], wq_d)
            nc.sync.dma_start(xall[:, 0:4, 0:512], xh_d[:, 0:4, 0:512])
            nc.sync.dma_start(xall[:, 4:8, 0:512], xh_d[:, 4:8, 0:512])
            nc.sync.dma_start(wk_sb[:], wk_d)
            nc.scalar.dma_start(xall[:, :, 512:1024], xh_d[:, :, 512:1024])
            nc.sync.dma_start(xall[:, :, 1024:2048], xh_d[:, :, 1024:2048])
            nc.scalar.dma_start(xall[:, :, 2048:3072], xh_d[:, :, 2048:3072])
            nc.sync.dma_start(xall[:, :, 3072:4096], xh_d[:, :, 3072:4096])

            # small/late loads on the gpsimd (SWDGE) queue
            nc.gpsimd.dma_start(bq8_sb[:], bq8_d)
            nc.gpsimd.dma_start(bk_sb[:], bk_d)

            # Q^T / K^T augmented per local head: [65, 4096]
            qt_sb = [acts.tile([HEAD_DIM + 1, NSEQ], bf16, tag=f"qt{h}", name=f"qt{h}") for h in range(HPC)]
            kt_sb = [acts.tile([HEAD_DIM + 1, NSEQ], bf16, tag=f"kt{h}", name=f"kt{h}") for h in range(HPC)]
            # b0 halves of the h0 augs are needed by the first scores tile
            nc.sync.dma_start(qt_sb[0][HEAD_DIM : HEAD_DIM + 1, :S], qaug_d[0:1, :S])
            nc.scalar.dma_start(kt_sb[0][HEAD_DIM : HEAD_DIM + 1, :S], kaug_d[0:1, :S])
            nc.gpsimd.dma_start(wv_sb[:], wv_d)
            nc.gpsimd.dma_start(qt_sb[1][HEAD_DIM : HEAD_DIM + 1, :S], qaug_d[1:2, :S])
            nc.gpsimd.dma_start(kt_sb[1][HEAD_DIM : HEAD_DIM + 1, :S], kaug_d[1:2, :S])
            for h in range(HPC):
                nc.gpsimd.dma_start(qt_sb[h][HEAD_DIM : HEAD_DIM + 1, S:], qaug_d[h : h + 1, S:])
                nc.gpsimd.dma_start(kt_sb[h][HEAD_DIM : HEAD_DIM + 1, S:], kaug_d[h : h + 1, S:])
            nc.gpsimd.dma_start(wo_sb[:], wo_d)

            # V_aug: [128(k), bh, kt, 65]; col 64 = ones (denominator trick)
            v_all = acts.tile([P, B * HPC, SCT, HEAD_DIM + 1], bf16, tag="vall")
            nc.vector.memset(v_all[:, :, :, HEAD_DIM : HEAD_DIM + 1], 1.0)
            # attention output (d-major), per batch
            ao_sb = [acts.tile([DC, S], bf16, tag=f"ao{b}", name=f"ao{b}") for b in range(B)]

            # ---- projection chunk emitters (n = 512-col chunk of seq) ----
            def proj_chunk(w_sb, post, n, t0=0, t1=KT, ps_box=[None]):
                if t0 == 0:
                    if PS_DR:
                        ps_box[0] = ps_dr.tile([P, 512], f32, tag="dr", name="ps_p")
                    else:
                        ps_box[0] = ps_sc.tile([P, 1024], f32, tag="mm", name="ps_p")
                pss = ps_box[0][:, :512]
                for t in range(t0, t1):
                    nc.tensor.matmul(
                        pss,
                        w_sb[:, t, :],
                        xall[:, t, n * 512 : (n + 1) * 512],
                        start=(t == 0),
                        stop=(t == KT - 1),
                    )
                if t1 == KT:
                    post(n, pss)

            def q_post(n, pss):
                for h in range(HPC):
                    nc.vector.tensor_scalar(
                        qt_sb[h][:HEAD_DIM, n * 512 : (n + 1) * 512],
                        pss[h * HEAD_DIM : (h + 1) * HEAD_DIM, :],
                        0.125,
                        bq8_sb[h * HEAD_DIM : (h + 1) * HEAD_DIM, :],
                        mult,
                        add,
                    )

            def k_post(n, pss):
                for h in range(HPC):
                    nc.vector.tensor_scalar_add(
                        kt_sb[h][:HEAD_DIM, n * 512 : (n + 1) * 512],
                        pss[h * HEAD_DIM : (h + 1) * HEAD_DIM, :],
                        bk_sb[h * HEAD_DIM : (h + 1) * HEAD_DIM, :],
                    )

            def proj_halves(w_sb, post, n):
                box = [None]
                return [
                    lambda: proj_chunk(w_sb, post, n, 0, 4, box),
                    lambda: proj_chunk(w_sb, post, n, 4, KT, box),
                ]

            # ---- V chunks: [k, dv] layout straight from the matmul ----
            def vnat_chunk(sc):
                def emit():
                    b, kt = divmod(sc, SCT)
                    if PS_DR:
                        ps = ps_dr.tile([P, 512], f32, tag="dr", name="ps_v")
                    else:
                        ps = ps_sc.tile([P, 1024], f32, tag="mm", name="ps_v")
                    pss = ps[:, :DC]
                    for t in range(KT):
                        nc.tensor.matmul(
                            pss,
                            xall[:, t, sc * P : (sc + 1) * P],
                            wv_sb[:, t, :],
                            start=(t == 0),
                            stop=(t == KT - 1),
                        )
                    nc.vector.tensor_copy(
                        v_all[:, b * HPC : (b + 1) * HPC, kt, :HEAD_DIM],
                        pss[:, :DC],
                    )

                return emit

            # ---- output projection chunks ----
            n_yd = [0]

            def oproj_chunk(b, ec, sc2):
                def emit():
                    # in the tail the scores pool is free: alternate pools for
                    # a deeper rotation so matmuls don't wait on copies
                    if not PS_DR or (tail[0] and n_yd[0] % 2 == 0):
                        ps = ps_sc.tile([P, 1024], f32, tag="mm", name="ps_o")
                    else:
                        ps = ps_dr.tile([P, 512], f32, tag="dr", name="ps_o")
                    pss = ps[:, :512]
                    nc.tensor.matmul(
                        pss,
                        wo_sb[:, ec * P : (ec + 1) * P],
                        ao_sb[b][:, sc2 * 512 : (sc2 + 1) * 512],
                        start=True,
                        stop=True,
                    )
                    y_sb = yp.tile([P, 512], bf16, tag="y", name="y_sb")
                    # only the post-attention tail may use the scalar engine
                    # (exp still owns it mid-kernel); alternate there
                    if (not TAILCP or tail[0]) and n_yd[0] % 2 == 0:
                        nc.scalar.copy(y_sb[:], pss)
                    else:
                        nc.vector.tensor_copy(y_sb[:], pss)
                    n_yd[0] += 1
                    dma_eng = (nc.sync, nc.gpsimd)[n_yd[0] % 2]
                    dma_eng.dma_start(
                        yt_d[
                            ec * P : (ec + 1) * P,
                            b * S + sc2 * 512 : b * S + (sc2 + 1) * 512,
                        ],
                        y_sb[:],
                    )

                return emit

            pending = []
            tail = [False]

            def drain(n=1):
                for _ in range(min(n, len(pending))):
                    pending.pop(0)()

            # ---- attention per (b, h, q-half) ----
            def attn_unit(b, h, qh, vjit=False, last=False, dr=None):
                col0 = b * S
                pv = ps_pv.tile([HEAD_DIM + 1, 1024], f32, tag="pv", name="pv")
                for kt in range(SCT):
                    if vjit:
                        vnat_chunk(b * SCT + kt)()
                    drain(dr[kt % len(dr)] if dr else 1)
                    ps = ps_sc.tile([P, 1024], f32, tag="mm", name="ps_s")
                    for c in range(2):
                        q0 = col0 + qh * 1024 + c * 512
                        nc.tensor.matmul(
                            ps[:, c * 512 : (c + 1) * 512],
                            kt_sb[h][:, col0 + kt * P : col0 + (kt + 1) * P],
                            qt_sb[h][:, q0 : q0 + 512],
                            start=True,
                            stop=True,
                        )
                    e = ep.tile([P, 1024], bf16, tag="e", name="e")
                    nc.scalar.activation(e[:], ps[:], Exp)
                    for c in range(2):
                        nc.tensor.matmul(
                            pv[:, c * 512 : (c + 1) * 512],
                            v_all[:, b * HPC + h, kt, :],
                            e[:, c * 512 : (c + 1) * 512],
                            start=(kt == 0),
                            stop=(kt == SCT - 1),
                        )
                # normalize: out = pv[0:64] / pv[64], in 512-col halves so
                # the matching oproj chunks unblock sooner.  Mid-kernel the
                # pv half is first copied to SBUF (frees the PSUM accumulator
                # for the next unit ~1.5us sooner); the last unit normalizes
                # straight from PSUM.
                srcs, rbs = [], []
                for hf in range(2):
                    cl = hf * 512
                    if last:
                        src = pv[:, cl : cl + 512]
                    else:
                        pvc = rp.tile([HEAD_DIM + 1, 512], f32, tag="pvc", name="pvc")
                        nc.vector.tensor_copy(pvc[:], pv[:, cl : cl + 512])
                        src = pvc[:]
                    srcs.append(src)
                    r_sb = rp.tile([1, 512], f32, tag="r", name="r_sb")
                    nc.vector.reciprocal(r_sb[:], src[HEAD_DIM : HEAD_DIM + 1, :])
                    rb = rp.tile([HEAD_DIM, 512], f32, tag="rb", name="rb")
                    nc.gpsimd.partition_broadcast(rb[:], r_sb[:])
                    rbs.append(rb)
                for hf in range(2):
                    cl = hf * 512
                    nc.vector.tensor_tensor(
                        ao_sb[b][
                            h * HEAD_DIM : (h + 1) * HEAD_DIM,
                            qh * 1024 + cl : qh * 1024 + cl + 512,
                        ],
                        srcs[hf][:HEAD_DIM, :],
                        rbs[hf][:],
                        mult,
                    )

            # ---- schedule ----
            if PREFIX == "min3i":
                # minimal prefix: attention unit0 starts after Q0,K0,Q1;
                # V(b0) is interleaved 1:1 with the remaining b0 projection
                # halves so unit0's drain(2) covers PV's JIT needs
                proj_chunk(wq_sb, q_post, 0)
                proj_chunk(wk_sb, k_post, 0)
                proj_chunk(wq_sb, q_post, 1)
                misc = []
                for n in (1, 2, 3):
                    misc.extend(proj_halves(wk_sb, k_post, n))
                for n in (2, 3):
                    misc.extend(proj_halves(wq_sb, q_post, n))
                for kt in range(SCT):
                    pending.append(vnat_chunk(kt))
                    if kt < len(misc):
                        pending.append(misc[kt])
                pending.extend(misc[SCT:])
                for sc in range(SCT, B * SCT):
                    pending.append(vnat_chunk(sc))
                for n in range(4, 8):
                    pending.extend(proj_halves(wq_sb, q_post, n))
                    pending.extend(proj_halves(wk_sb, k_post, n))
            elif PREFIX == "min3":
                proj_chunk(wq_sb, q_post, 0)
                proj_chunk(wk_sb, k_post, 0)
                proj_chunk(wq_sb, q_post, 1)
                for n in (1, 2, 3):
                    pending.extend(proj_halves(wk_sb, k_post, n))
                for n in (2, 3):
                    pending.extend(proj_halves(wq_sb, q_post, n))
            else:
                proj_chunk(wq_sb, q_post, 0)
                proj_chunk(wk_sb, k_post, 0)
                proj_chunk(wq_sb, q_post, 1)
                proj_chunk(wk_sb, k_post, 1)
                proj_chunk(wk_sb, k_post, 2)
                proj_chunk(wk_sb, k_post, 3)
                proj_chunk(wq_sb, q_post, 2)
                proj_chunk(wq_sb, q_post, 3)
            if PREFIX != "min3i":
                # V chunks FIRST: unit0's kt loop drains vnat(b0, kt) exactly
                # one iteration ahead of the PV matmul that consumes it
                if not VJIT:
                    for sc in range(B * SCT):
                        pending.append(vnat_chunk(sc))
                for n in range(4, 8):
                    pending.extend(proj_halves(wq_sb, q_post, n))
                    pending.extend(proj_halves(wk_sb, k_post, n))

            # drain cadence per unit: early units have mandatory JIT V
            # chunks; later units smooth the remaining fill work over the
            # Act-bound attention steady state (~186ns/kt deficit)
            CAD = opts.get("cad") or [
                [1], [1], [1, 0], [1, 0],
                [1, 1, 1, 0], [1, 1, 1, 0], [1, 1, 1, 0], [1, 1, 1, 0],
            ]
            ui = 0
            for b in range(B):
                for qh in range(2):
                    for h in range(HPC):
                        first = b == 0 and qh == 0 and h == 0
                        attn_unit(b, h, qh, vjit=VJIT and (qh == 0 and h == 0),
                                  last=(b == B - 1 and qh == 1 and h == HPC - 1),
                                  dr=[2] if (PREFIX == "min3i" and first) else CAD[min(ui, 7)])
                        ui += 1
                    # both heads of this q-half done -> its oproj chunks
                    # (sc2-major so tail chunks gated on the first normalize
                    # half don't queue behind second-half chunks)
                    pending.extend(
                        oproj_chunk(b, ec, sc2)
                        for sc2 in (2 * qh, 2 * qh + 1)
                        for ec in range(D_MODEL // P)
                    )
            tail[0] = True
            drain(len(pending))

    nc.compile()
    return nc


def _get_module(repeat=1, opts=None):
    key = f"nc{repeat}{opts}"
    if key not in _CACHE:
        _CACHE[key] = _build_module(repeat, opts)
    return _CACHE[key]


def _host_prep(x, temporal_features, wq, bq, wk, bk, wv, bv, wo, bo, wc, bc, cycle_scale):
    """Shard/lay out the inputs for the 8 cores."""
    x = np.asarray(x, np.float32)
    # [d, n] -> SBUF layout [p, t, n] with d = t*128 + p
    xt = np.ascontiguousarray(
        x.reshape(NSEQ, D_MODEL).T.reshape(8, P, NSEQ).transpose(1, 0, 2)
    ).astype(BF16)

    # cycle embedding (tiny): [B, S, H] -> [B, H, S], seq-normalized
    ce = (
        np.asarray(temporal_features, np.float32).reshape(NSEQ, -1) @ np.asarray(wc, np.float32).T
        + np.asarray(bc, np.float32)
    ).reshape(B, S, N_HEADS).transpose(0, 2, 1)
    nrm = np.maximum(np.linalg.norm(ce, axis=-1, keepdims=True), EPS)
    cn = ce / nrm  # [B, H, S]
    cs = np.asarray(cycle_scale, np.float32)

    def pack_w(w_rows):  # [DC, D] slice of W -> SBUF layout [p, t, m]
        wt = np.ascontiguousarray(w_rows.T)  # [D, DC]
        return np.ascontiguousarray(
            wt.reshape(8, P, DC).transpose(1, 0, 2)
        ).astype(BF16)

    in_maps = []
    for c in range(N_CORES):
        rows = slice(c * DC, (c + 1) * DC)
        qaug = np.empty((HPC, NSEQ), np.float32)
        kaug = np.empty((HPC, NSEQ), np.float32)
        for h in range(HPC):
            gh = c * HPC + h
            for b in range(B):
                qaug[h, b * S : (b + 1) * S] = cs[gh] * cn[b, gh]
                kaug[h, b * S : (b + 1) * S] = cn[b, gh]
        in_maps.append(
            {
                "xh": xt,
                "wq_p": pack_w(np.asarray(wq, np.float32)[rows]),
                "wk_p": pack_w(np.asarray(wk, np.float32)[rows]),
                "wv_p": pack_w(np.asarray(wv, np.float32)[rows]),
                "wo_t": np.ascontiguousarray(np.asarray(wo, np.float32)[:, rows].T).astype(BF16),
                "qaug": qaug.astype(BF16),
                "kaug": kaug.astype(BF16),
                "bq8": (np.asarray(bq, np.float32)[rows] * 0.125).reshape(DC, 1).copy(),
                "bk": np.asarray(bk, np.float32)[rows].reshape(DC, 1).copy(),
            }
        )
    return in_maps


def kernel(**inputs):
    from concourse import bass_utils

    nc = _get_module()
    in_maps = _host_prep(**inputs)
    res = bass_utils.run_bass_kernel_spmd(nc, in_maps, core_ids=list(range(N_CORES)))
    yt = np.zeros((D_MODEL, NSEQ), np.float64)
    for r in res.results:
        yt += r["yt"].astype(np.float64)
    # bv is folded out of the device kernel: attn rows sum to 1, so
    # attn@(V+bv) @ wo.T = attn@V @ wo.T + bv @ wo.T
    bias = np.asarray(inputs["bo"], np.float64) + np.asarray(
        inputs["bv"], np.float64
    ) @ np.asarray(inputs["wo"], np.float64).T
    y = yt.T.reshape(B, S, D_MODEL) + bias
    return y.astype(np.float32)
